# revision 2
# baseline (speedup 1.0000x reference)
"""LRU (Linear Recurrent Unit) single-step forward on 8 Trainium2 NeuronCores.

Math: with seq-len 1 the whole LRU step collapses algebraically to one GEMM:
    y[b,:] = W @ u[b] + bias
where
    W    = 2*C_re@diag(g)@B_re - 2*C_im@diag(g)@B_im + D          [DOUT, DIN]
    bias = 2*(C_re@(lam_re*x_re - lam_im*x_im)
              - C_im@(lam_re*x_im + lam_im*x_re))                  [DOUT]
    g = exp(gamma_log), lam = exp(-exp(nu_log)) * exp(i*exp(theta_log)).

The parameter fold (W, bias) is input-data independent (depends only on the
model parameters / initial state), computed once on host in float64.  The
batch GEMM (99% of FLOPs and bytes) runs on the 8 NeuronCores, data-parallel
over the batch: each core computes y_shard^T = W @ u_shard^T (+bias).
"""

import numpy as np

BATCH, DIN, DSTATE, DOUT = 16384, 1024, 2048, 1024
N_CORES = 8
B_SHARD = BATCH // N_CORES  # 2048 rows per core
P = 128                     # SBUF partitions
NB = 512                    # batch tile (moving free dim, max 512)
I_BLOCKS = DIN // P         # 8 contraction blocks
J_BLOCKS = DOUT // P        # 8 output-row blocks
B_TILES = B_SHARD // NB     # 4 batch tiles per core

_CACHE = {}


def _build_nc():
    import concourse.mybir as mybir
    import concourse.tile as tile
    from concourse import bacc
    from concourse._compat import get_trn_type

    nc = bacc.Bacc(get_trn_type() or "TRN2", target_bir_lowering=False)
    f32 = mybir.dt.float32
    f32r = mybir.dt.float32r  # full-rate fp32 matmul mode on TRN2

    ut = nc.declare_dram_parameter("ut", [DIN, B_SHARD], f32r, isOutput=False)
    wt = nc.declare_dram_parameter("wt", [DIN, DOUT], f32r, isOutput=False)
    bias = nc.declare_dram_parameter("bias", [P, J_BLOCKS], f32, isOutput=False)
    yt = nc.declare_dram_parameter("yt", [DOUT, B_SHARD], f32, isOutput=True)

    with tile.TileContext(nc) as tc:
        with (
            tc.tile_pool(name="consts", bufs=1) as consts,
            tc.tile_pool(name="upool", bufs=3) as upool,
            tc.tile_pool(name="opool", bufs=4) as opool,
            tc.tile_pool(name="psum", bufs=4, space="PSUM") as psum,
        ):
            bias_t = consts.tile([P, J_BLOCKS], f32, tag="bias")
            nc.sync.dma_start(out=bias_t[:], in_=bias[:])
            w_tiles = []
            for ib in range(I_BLOCKS):
                w_t = consts.tile([P, DOUT], f32r, tag=f"w{ib}", name=f"w{ib}")
                nc.sync.dma_start(out=w_t[:], in_=wt[ib * P:(ib + 1) * P, :])
                w_tiles.append(w_t)

            for bt in range(B_TILES):
                u_tiles = []
                for ib in range(I_BLOCKS):
                    u_t = upool.tile([P, NB], f32r, tag=f"u{ib}", name=f"u{ib}_{bt}")
                    nc.sync.dma_start(
                        out=u_t[:],
                        in_=ut[ib * P:(ib + 1) * P, bt * NB:(bt + 1) * NB],
                    )
                    u_tiles.append(u_t)
                for jb in range(J_BLOCKS):
                    pt = psum.tile([P, NB], f32, tag="pt", name=f"pt_{bt}_{jb}")
                    for ib in range(I_BLOCKS):
                        nc.tensor.matmul(
                            pt[:],
                            w_tiles[ib][:, jb * P:(jb + 1) * P],
                            u_tiles[ib][:],
                            start=(ib == 0),
                            stop=(ib == I_BLOCKS - 1),
                        )
                    ot = opool.tile([P, NB], f32, tag="ot", name=f"ot_{bt}_{jb}")
                    nc.scalar.activation(
                        ot[:], pt[:], mybir.ActivationFunctionType.Identity,
                        bias=bias_t[:, jb:jb + 1],
                    )
                    nc.sync.dma_start(
                        out=yt[jb * P:(jb + 1) * P, bt * NB:(bt + 1) * NB],
                        in_=ot[:],
                    )
    nc.compile()
    return nc


def _fold_params(x_re, x_im, nu_log, theta_log, gamma_log, B_re, B_im, C_re, C_im, D):
    """Fold the LRU parameters into (W^T [DIN, DOUT], bias [DOUT]) in float64."""
    nu = np.asarray(nu_log, np.float64)
    th = np.exp(np.asarray(theta_log, np.float64))
    lam_mod = np.exp(-np.exp(nu))
    lam_re = lam_mod * np.cos(th)
    lam_im = lam_mod * np.sin(th)
    g = np.exp(np.asarray(gamma_log, np.float64))
    C_re64 = np.asarray(C_re, np.float64)
    C_im64 = np.asarray(C_im, np.float64)
    W = (2.0 * ((C_re64 * g) @ np.asarray(B_re, np.float64))
         - 2.0 * ((C_im64 * g) @ np.asarray(B_im, np.float64))
         + np.asarray(D, np.float64))  # [DOUT, DIN]
    xr = np.asarray(x_re, np.float64)
    xi = np.asarray(x_im, np.float64)
    lx_re = lam_re * xr - lam_im * xi
    lx_im = lam_re * xi + lam_im * xr
    bias = 2.0 * (C_re64 @ lx_re - C_im64 @ lx_im)  # [DOUT]
    return W.T.astype(np.float32).copy(), bias.astype(np.float32)


def kernel(u_in, x_re, x_im, nu_log, theta_log, gamma_log, B_re, B_im,
           C_re, C_im, D, _trace=False):
    from concourse.bass_utils import run_bass_kernel_spmd

    wt_host, bias_host = _fold_params(
        x_re, x_im, nu_log, theta_log, gamma_log, B_re, B_im, C_re, C_im, D)
    bias2 = np.ascontiguousarray(bias_host.reshape(J_BLOCKS, P).T)  # [128, 8]

    u2 = np.asarray(u_in, np.float32).reshape(BATCH, DIN)
    core_ids = list(range(N_CORES))
    in_maps = []
    for c in core_ids:
        shard = u2[c * B_SHARD:(c + 1) * B_SHARD]          # [2048, 1024]
        in_maps.append({
            "ut": np.ascontiguousarray(shard.T),           # [1024, 2048]
            "wt": wt_host,
            "bias": bias2,
        })

    if "nc" not in _CACHE:
        _CACHE["nc"] = _build_nc()
    res = run_bass_kernel_spmd(_CACHE["nc"], in_maps, core_ids, trace=_trace)

    y = np.empty((BATCH, DOUT), np.float32)
    for c in core_ids:
        y[c * B_SHARD:(c + 1) * B_SHARD] = res.results[c]["yt"].T
    out = y.reshape(BATCH, 1, DOUT)
    if _trace:
        return out, res
    return out
